# revision 81
# baseline (speedup 1.0000x reference)
"""BiMamba (fwd+bwd Mamba2 + fusion Linear) Trainium2 kernel.

Sharding: 8 cores = 2 branches x 4 batches. Each core runs one full Mamba2
branch on one batch element via the chunked SSD formulation (chunk=128), with
the out-proj and fusion Linear folded into one matmul (W_comb). Host flips x
for the backward branch and sums the two per-branch partial outputs.

Optimizations over the first working version (1.265ms -> ~1.09ms):
- Y_inter and the per-head Y_intra matmuls accumulate into one PSUM bank ring
  (in-place decay scale between accumulation groups) instead of separate
  PSUM tiles + DVE adds; the state-update and out-proj matmuls reuse the ring.
- rsqrt for the RMS norm runs on the DVE (bit hack + Newton), and softplus/ln
  stay paired with exp, so the scalar engine needs ~2 act-table loads per
  L-tile instead of 3 per chunk (was 124us of table thrash).
- The RMS-norm scale is folded into the out-proj PSUM evacuation (tensor_scalar
  with the per-token rstd), removing a full-width normalize pass.
- Conv tiles are transposed to token layout pre-SiLU as soon as each conv
  m-tile finishes (overlapping the conv phase); SiLU is applied per-chunk on
  the transposed tile, removing the L-tile-start serialization.
- Software-pipelined emission: the next L-tile's in_proj/conv/dt matmuls are
  emitted between the current L-tile's chunks (engine queues are in-order, so
  emission order controls overlap); feeds run two chunks ahead through a
  triple-buffered mask-operand ring.
- Chunk-invariant pieces (decay transposes, causal-masked G^T, B token layout)
  are precomputed one chunk ahead, off the state-recurrence spine.

Engine notes (hardware-measured): GpSimd elementwise offload is a trap (Q7
SBUF traffic slows DVE/Scalar ~1.5x); a single fp32 matmul inserted in the
bf16 stream costs ~70us in PE mode switches; scalar-engine ops run ~2x the
documented cycle model on this part.
"""

import sys

sys.path.insert(0, "/opt/trn_rl_repo")

import numpy as np
import ml_dtypes

D_MODEL = 768
D_STATE = 64
D_CONV = 4
D_INNER = 1536
HEADDIM = 64
H = 24
CONV_DIM = D_INNER + 2 * D_STATE  # 1664
D_IN_PROJ = 2 * D_INNER + 2 * D_STATE + H  # 3224
BATCH, SEQ = 4, 4096

LT = 512  # L-tile
NLT = SEQ // LT  # 8
Q = 128  # chunk
NCH = LT // Q  # chunks per L-tile
KT = D_MODEL // 128  # 6 k-tiles
MX = 13  # xBC m-tiles (1664/128)
NZ = D_INNER // 512  # 3 z slices
BF = "bfloat16"

_CACHE = {}


def _build_nc():
    import concourse.bass as bass
    import concourse.bacc as bacc
    import concourse.mybir as mybir
    from concourse.tile import TileContext
    from concourse.masks import make_identity

    fp32 = mybir.dt.float32
    bf16 = mybir.dt.bfloat16
    AX = mybir.AluOpType
    ACT = mybir.ActivationFunctionType

    nc = bacc.Bacc("TRN2", debug=False, num_devices=8)

    xT = nc.declare_dram_parameter("xT", [D_MODEL, SEQ], bf16, isOutput=False)
    wip = nc.declare_dram_parameter("wip", [D_MODEL, D_IN_PROJ], bf16, isOutput=False)
    wcb = nc.declare_dram_parameter("wcb", [D_INNER, D_MODEL], bf16, isOutput=False)
    cw = nc.declare_dram_parameter("cw", [CONV_DIM, D_CONV], fp32, isOutput=False)
    cb = nc.declare_dram_parameter("cb", [CONV_DIM], fp32, isOutput=False)
    dtb = nc.declare_dram_parameter("dtb", [H, 1], fp32, isOutput=False)
    apos = nc.declare_dram_parameter("apos", [H, 1], fp32, isOutput=False)
    dsb = nc.declare_dram_parameter("dsb", [128, D_INNER], bf16, isOutput=False)
    ind2 = nc.declare_dram_parameter("ind2", [88, H * Q], bf16, isOutput=False)
    out = nc.declare_dram_parameter("out", [SEQ, D_MODEL], bf16, isOutput=True)

    with TileContext(nc) as tc:
        with (
            tc.tile_pool(name="const", bufs=1) as cpool,
            tc.tile_pool(name="xt", bufs=2) as xtpool,
            tc.tile_pool(name="work", bufs=2) as wpool,
            tc.tile_pool(name="conv", bufs=2) as convpool,
            tc.tile_pool(name="convin", bufs=1) as cinpool,
            tc.tile_pool(name="ssd", bufs=2) as spool,
            tc.tile_pool(name="ssdsm", bufs=3) as smpool,
            tc.tile_pool(name="state", bufs=1) as statepool,
            tc.tile_pool(name="pbig", bufs=2, space="PSUM") as pbig,
            tc.tile_pool(name="pbc", bufs=2, space="PSUM") as pbcp,
            tc.tile_pool(name="psm", bufs=1, space="PSUM") as psm,
            tc.tile_pool(name="pyacc", bufs=3, space="PSUM") as pyacc,
        ):
            # ---- constants ----
            wip_sb = cpool.tile([128, KT, D_IN_PROJ], bf16, tag="wip")
            nc.sync.dma_start(
                out=wip_sb[:], in_=wip.ap().rearrange("(k p) m -> p k m", p=128)
            )
            wcb_sb = cpool.tile([128, 12, D_MODEL], bf16, tag="wcb")
            nc.sync.dma_start(
                out=wcb_sb[:], in_=wcb.ap().rearrange("(j p) m -> p j m", p=128)
            )
            cw_sb = cpool.tile([128, MX, D_CONV], fp32, tag="cw")
            nc.sync.dma_start(
                out=cw_sb[:], in_=cw.ap().rearrange("(a p) k -> p a k", p=128)
            )
            cb_sb = cpool.tile([128, MX], fp32, tag="cb")
            nc.sync.dma_start(
                out=cb_sb[:], in_=cb.ap().rearrange("(a p) -> p a", p=128)
            )
            dtb_sb = cpool.tile([H, 1], fp32, tag="dtb")
            nc.sync.dma_start(out=dtb_sb[:], in_=dtb.ap())
            apos_sb = cpool.tile([H, 1], fp32, tag="apos")
            nc.sync.dma_start(out=apos_sb[:], in_=apos.ap())
            dsb_sb = cpool.tile([128, D_INNER], bf16, tag="dsb")
            nc.sync.dma_start(out=dsb_sb[:], in_=dsb.ap())
            rhs88p = []
            for pi in range(3):
                r8 = cpool.tile([88, H * Q], bf16, tag=f"rhs88_{pi}")
                nc.sync.dma_start(out=r8[:], in_=ind2.ap())
                rhs88p.append(r8)

            ident_f64 = cpool.tile([64, 64], fp32, tag="idf64")
            make_identity(nc, ident_f64[:])
            ident_b128 = cpool.tile([128, 128], bf16, tag="idb")
            make_identity(nc, ident_b128[:])
            bT88p = []
            for pi in range(3):
                b8 = cpool.tile([88, 128], bf16, tag=f"bT88_{pi}")
                nc.gpsimd.memset(b8[:], 0.0)
                nc.gpsimd.memset(b8[0:2, :], -1.0)
                bT88p.append(b8)
            pones2 = cpool.tile([2, 64], bf16, tag="pones2")
            nc.gpsimd.memset(pones2[:], 1.0)
            zer24 = cpool.tile([H, 128], bf16, tag="zer24")
            nc.gpsimd.memset(zer24[:], 0.0)

            # ---- loop-carried state ----
            S_f = statepool.tile([64, H * 64], fp32, tag="Sf")
            nc.vector.memset(S_f[:], 0.0)
            S_b = statepool.tile([64, H * 64], bf16, tag="Sb")
            nc.vector.memset(S_b[:], 0.0)

            halo = [
                convpool.tile([128, 3], bf16, tag=f"halo{m}", name=f"halo{m}")
                for m in range(MX)
            ]
            for m in range(MX):
                nc.vector.memset(halo[m][:], 0.0)

            lctx = {}

            def emit_head(li):
                l0 = li * LT
                xtt = xtpool.tile([128, KT, LT], bf16, tag="xtt")
                nc.sync.dma_start(
                    out=xtt[:],
                    in_=xT.ap().rearrange("(k p) l -> p k l", p=128)[:, :, l0 : l0 + LT],
                )
                # bufs=1: each conv tile is consumed (transposed / B-C copied)
                # before the next L-tile's conv overwrites it
                co_all = convpool.tile([128, MX, LT], bf16, tag="co", bufs=1)
                # token-major x (pre-SiLU); filled by per-m transposes during conv
                xstLT = wpool.tile([128, NCH, D_INNER], bf16, tag="xstLT", bufs=2)
                lctx[li] = {"xtt": xtt, "co": co_all, "xstLT": xstLT}

            def emit_conv(li, ms):
                ctx = lctx[li]
                xtt, co_all = ctx["xtt"], ctx["co"]
                for m in ms:
                    ps = pbig.tile([128, LT], fp32, tag="big")
                    for k in range(KT):
                        nc.tensor.matmul(
                            ps[:],
                            lhsT=wip_sb[:, k, D_INNER + m * 128 : D_INNER + (m + 1) * 128],
                            rhs=xtt[:, k, :],
                            start=(k == 0),
                            stop=(k == KT - 1),
                        )
                    xin = cinpool.tile([128, LT + 3], bf16, tag="xin", bufs=2)
                    nc.any.tensor_copy(xin[:, 0:3], halo[m][:])
                    nc.any.tensor_copy(xin[:, 3 : LT + 3], ps[:])
                    nh = convpool.tile([128, 3], bf16, tag=f"halo{m}")
                    nc.any.tensor_copy(nh[:], xin[:, LT : LT + 3])
                    halo[m] = nh
                    # conv as 4 independent TS products (4x DVE mode) + add tree
                    # (all on DVE: cross-engine splits of this chain cause SBUF
                    # contention that slows both engines)
                    co = co_all[:, m, :]
                    t0 = cinpool.tile([128, LT], bf16, tag="t0", bufs=2)
                    nc.vector.tensor_scalar(
                        out=t0[:], in0=xin[:, 3 : LT + 3],
                        scalar1=cw_sb[:, m, 3:4], scalar2=cb_sb[:, m : m + 1],
                        op0=AX.mult, op1=AX.add,
                    )
                    t1 = cinpool.tile([128, LT], bf16, tag="t1", bufs=2)
                    nc.vector.tensor_scalar(
                        out=t1[:], in0=xin[:, 2 : LT + 2],
                        scalar1=cw_sb[:, m, 2:3], scalar2=None, op0=AX.mult,
                    )
                    nc.vector.tensor_tensor(out=t0[:], in0=t0[:], in1=t1[:], op=AX.add)
                    nc.vector.tensor_scalar(
                        out=t1[:], in0=xin[:, 1 : LT + 1],
                        scalar1=cw_sb[:, m, 1:2], scalar2=None, op0=AX.mult,
                    )
                    t2 = cinpool.tile([128, LT], bf16, tag="t2", bufs=2)
                    nc.vector.tensor_scalar(
                        out=t2[:], in0=xin[:, 0:LT],
                        scalar1=cw_sb[:, m, 0:1], scalar2=None, op0=AX.mult,
                    )
                    nc.vector.tensor_tensor(out=t1[:], in0=t1[:], in1=t2[:], op=AX.add)
                    nc.vector.tensor_tensor(out=co, in0=t0[:], in1=t1[:], op=AX.add)
                    # transpose this conv tile to token layout NOW (pre-SiLU) so
                    # the xbar DMAs overlap the remaining conv work; SiLU is
                    # applied per-chunk on the transposed tile instead
                    if m < 12:
                        nc.sync.dma_start_transpose(
                            out=ctx["xstLT"][:, :, m * 128 : (m + 1) * 128],
                            in_=co,
                        )

            def emit_bc_silu(li):
                # B/C conv tile SiLU (channel layout) + per-chunk SiLU on
                # token-major x + z-gate matmuls/SiLU: one contiguous SiLU
                # block per L-tile keeps the act-table switches at ~2/L-tile
                ctx = lctx[li]
                xtt, co_all = ctx["xtt"], ctx["co"]
                xstLT = ctx["xstLT"]
                nc.scalar.activation(co_all[:, 12, :], co_all[:, 12, :], ACT.Silu)
                Bt = convpool.tile([64, LT], bf16, tag="Bt", bufs=1)
                nc.any.tensor_copy(Bt[:], co_all[0:64, 12, :])
                Ct = convpool.tile([64, LT], bf16, tag="Ct", bufs=1)
                nc.sync.dma_start(out=Ct[:], in_=co_all[64:128, 12, :])
                for c4 in range(NCH):
                    nc.scalar.activation(
                        xstLT[:, c4, :], xstLT[:, c4, :], ACT.Silu
                    )
                sz_list = []
                for c4 in range(NCH):
                    c4s = slice(c4 * Q, (c4 + 1) * Q)
                    sz_c = wpool.tile([128, D_INNER], bf16, tag="szlt", bufs=4)
                    for j in range(NZ):
                        pz = pbig.tile([128, 512], fp32, tag="big")
                        for k in range(KT):
                            nc.tensor.matmul(
                                pz[:],
                                lhsT=xtt[:, k, c4s],
                                rhs=wip_sb[:, k, j * 512 : (j + 1) * 512],
                                start=(k == 0), stop=(k == KT - 1),
                            )
                        nc.scalar.activation(
                            sz_c[:, j * 512 : (j + 1) * 512], pz[:], ACT.Silu
                        )
                    sz_list.append(sz_c)
                ctx.update(Bt=Bt, Ct=Ct, sz_list=sz_list)

            def emit_dt(li):
                ctx = lctx[li]
                xtt = ctx["xtt"]
                # ---- in_proj: dt part (channel layout, 24 rows) ----
                psdt = pbig.tile([H, LT], fp32, tag="big")
                for k in range(KT):
                    nc.tensor.matmul(
                        psdt[:],
                        lhsT=wip_sb[:, k, D_INNER + CONV_DIM : D_IN_PROJ],
                        rhs=xtt[:, k, :],
                        start=(k == 0),
                        stop=(k == KT - 1),
                    )
                # dt = softplus(psdt + dt_bias) via exp + ln1p (both in one act table)
                art0 = wpool.tile([H, LT], fp32, tag="art0", bufs=1)
                nc.scalar.activation(art0[:], psdt[:], ACT.Exp, bias=dtb_sb[:])
                dtt = wpool.tile([H, LT], fp32, tag="dtt", bufs=1)
                nc.scalar.activation(dtt[:], art0[:], ACT.Ln, bias=1.0)
                # a*dt and its within-chunk cumsum
                art = wpool.tile([H, LT], fp32, tag="art", bufs=1)
                nc.vector.tensor_scalar(
                    out=art[:], in0=dtt[:], scalar1=apos_sb[:],
                    scalar2=None, op0=AX.mult,
                )
                cnt = wpool.tile([H, LT], fp32, tag="cnt", bufs=1)
                for c in range(NCH):
                    cs = slice(c * Q, (c + 1) * Q)
                    nc.vector.tensor_tensor_scan(
                        out=cnt[:, cs],
                        data0=art[:, cs],
                        data1=zer24[:],
                        initial=0.0,
                        op0=AX.add,
                        op1=AX.add,
                    )
                # ln(dt) head-major for the mask-bias matmul
                lnb = wpool.tile([H, LT], fp32, tag="lnb", bufs=1)
                nc.scalar.activation(lnb[:], dtt[:], ACT.Ln)
                nc.vector.tensor_tensor(out=lnb[:], in0=lnb[:], in1=cnt[:], op=AX.add)

                hi24 = wpool.tile([H, LT], bf16, tag="hi24", bufs=1)
                nc.any.tensor_copy(hi24[:], cnt[:])
                lo24 = wpool.tile([H, LT], bf16, tag="lo24", bufs=1)
                nc.vector.tensor_sub(lo24[:], cnt[:], hi24[:])
                ctx.update(dtt=dtt, cnt=cnt, lnb=lnb, hi24=hi24, lo24=lo24)

                # ---- per-L-tile precompute of all chunk-invariant SSD pieces
                # (off the per-chunk state-recurrence spine) ----
                # wv for all chunks: exp(cnt - chunk_end_cnt)
                cend = wpool.tile([H, NCH], fp32, tag="cend", bufs=1)
                nc.vector.tensor_copy(
                    cend[:], cnt[:].rearrange("p (c q) -> p c q", c=NCH)[:, :, Q - 1]
                )
                wvLT = wpool.tile([H, LT], fp32, tag="wvLT", bufs=1)
                nc.vector.tensor_tensor(
                    out=wvLT[:].rearrange("p (c q) -> p c q", c=NCH),
                    in0=cnt[:].rearrange("p (c q) -> p c q", c=NCH),
                    in1=cend[:].unsqueeze(2).broadcast_to([H, NCH, Q]),
                    op=AX.subtract,
                )
                nc.scalar.activation(wvLT[:], wvLT[:], ACT.Exp)
                # stacked [wdt ; pad ; cnt] (64, LT): one build for all chunks
                stkLT = wpool.tile([64, LT], fp32, tag="stkLT", bufs=1)
                nc.gpsimd.memset(stkLT[:], 0.0)
                nc.vector.tensor_tensor(
                    out=stkLT[0:H, :], in0=dtt[:], in1=wvLT[:], op=AX.mult
                )
                nc.vector.tensor_copy(stkLT[32 : 32 + H, :], cnt[:])
                ctx.update(stkLT=stkLT, wdtTs={}, expcums={}, gtm4s={}, btoks={})

            def emit_pre(li, c):
                # chunk-invariant SSD pieces for chunk c, emitted one chunk
                # ahead (pipelined off the state-recurrence spine)
                ctx = lctx[li]
                Bt, Ct, stkLT = ctx["Bt"], ctx["Ct"], ctx["stkLT"]
                cs = slice(c * Q, (c + 1) * Q)
                pwc = psm.tile([128, 64], fp32, tag="sm2")
                nc.tensor.transpose(pwc[:], stkLT[:, cs], ident_f64[:])
                wdtT = smpool.tile([128, H], bf16, tag="wdtT", bufs=4)
                nc.vector.tensor_copy(wdtT[:], pwc[:, 0:H])
                expcum = smpool.tile([128, H], fp32, tag="expcum", bufs=4)
                nc.scalar.activation(
                    expcum[:], pwc[:, 32 : 32 + H], ACT.Exp, scale=-1.0
                )
                # G^T with causal mask + 4-head broadcast
                pgt = psm.tile([128, Q], fp32, tag="sm2")
                nc.tensor.matmul(
                    pgt[:], lhsT=Bt[:, cs], rhs=Ct[:, cs], start=True, stop=True
                )
                gtc = smpool.tile([128, Q], bf16, tag="gtc", bufs=4)
                nc.vector.tensor_copy(gtc[:], pgt[:])
                gtm4 = smpool.tile([128, 4, Q], bf16, tag="gtm4", bufs=4)
                nc.gpsimd.affine_select(
                    out=gtm4[:],
                    in_=gtc[:].unsqueeze(1).broadcast_to([128, 4, Q]),
                    compare_op=AX.is_ge, fill=0.0,
                    base=0, pattern=[[0, 4], [1, Q]], channel_multiplier=-1,
                )
                # B token layout
                pbt = psm.tile([128, 64], bf16, tag="sm2")
                nc.tensor.transpose(pbt[:], Bt[:, cs], ident_b128[0:64, 0:64])
                btok = smpool.tile([128, 64], bf16, tag="btok", bufs=4)
                nc.vector.tensor_copy(btok[:], pbt[:])
                ctx["wdtTs"][c] = wdtT; ctx["expcums"][c] = expcum
                ctx["gtm4s"][c] = gtm4; ctx["btoks"][c] = btok

            def emit_feed(li, c):
                ctx = lctx[li]
                lnb, hi24, lo24 = ctx["lnb"], ctx["hi24"], ctx["lo24"]
                cs = slice(c * Q, (c + 1) * Q)
                rhs88 = rhs88p[(li * NCH + c) % 3]
                bT88 = bT88p[(li * NCH + c) % 3]
                h4 = rhs88[0:2, :].rearrange("p (h t) -> p h t", h=H)
                nc.sync.dma_start(out=h4[0:1, :, :], in_=hi24[:, cs])
                nc.sync.dma_start(out=h4[1:2, :, :], in_=lo24[:, cs])
                # bias rows (bf16 hi/lo of ln(dt)+cumneg), head-major
                bhi = smpool.tile([H, 128], bf16, tag="bhi")
                nc.any.tensor_copy(bhi[:], lnb[:, cs])
                blo = smpool.tile([H, 128], bf16, tag="blo")
                nc.vector.tensor_sub(blo[:], lnb[:, cs], bhi[:])
                nc.sync.dma_start(out=bT88[32 : 32 + H, :], in_=bhi[:])
                nc.sync.dma_start(out=bT88[64 : 64 + H, :], in_=blo[:])

            def emit_chunk(li, c, tail_feed=None, mid_cb=None, pending=None):
                ctx = lctx[li]
                Bt, Ct, sz_list = ctx["Bt"], ctx["Ct"], ctx["sz_list"]
                dtt, cnt = ctx["dtt"], ctx["cnt"]
                l0 = li * LT
                cs = slice(c * Q, (c + 1) * Q)
                lend = c * Q + Q - 1
                rhs88 = rhs88p[(li * NCH + c) % 3]
                bT88 = bT88p[(li * NCH + c) % 3]
                xst = ctx["xstLT"][:, c, :]
                B_ch = Bt[:, cs]
                C_ch = Ct[:, cs]

                emit_pre(li, c)
                wdtT = ctx["wdtTs"][c]
                expcum = ctx["expcums"][c]
                gtm4 = ctx["gtm4s"][c]
                btok = ctx["btoks"][c]

                # chunk decay factors (64, 24) from the feed rows
                pcd = psm.tile([64, H], fp32, tag="sm2")
                nc.tensor.matmul(
                    pcd[:], lhsT=pones2[:], rhs=rhs88[0:2, :].rearrange("p (h t) -> p h t", h=H)[:, :, Q - 1],
                    start=True, stop=True,
                )
                cdec = smpool.tile([64, H], fp32, tag="cdec")
                nc.scalar.activation(cdec[:], pcd[:], ACT.Exp, scale=-1.0)

                # wxd = wdt * x (token layout). NOTE: GpSimd elementwise is a
                # trap — Q7 SBUF traffic contends with the DVE/Scalar ports and
                # slows every other engine ~1.5x. Keep on DVE.
                wdt_b = wdtT[:].unsqueeze(2).broadcast_to([128, H, 64])
                wxd = spool.tile([128, H, 64], bf16, tag="wxd")
                nc.vector.tensor_tensor(
                    out=wxd[:],
                    in0=xst.rearrange("p (h d) -> p h d", h=H),
                    in1=wdt_b, op=AX.mult,
                )
                # state decay (in place)
                cdb = cdec[:].unsqueeze(2).broadcast_to([64, H, 64])
                nc.vector.tensor_tensor(
                    out=S_f[:].rearrange("p (h d) -> p h d", h=H),
                    in0=S_f[:].rearrange("p (h d) -> p h d", h=H),
                    in1=cdb, op=AX.mult,
                )

                # dxs = D * x (skip term)
                dxs = spool.tile([128, D_INNER], bf16, tag="dxs", bufs=1)
                nc.vector.tensor_tensor(out=dxs[:], in0=xst, in1=dsb_sb[:], op=AX.mult)

                ywork = spool.tile([128, D_INNER], bf16, tag="ywork")
                ecb = expcum[:].unsqueeze(2).broadcast_to([128, H, 64])

                # ---- Y accumulation: inter (scaled in place) + intra heads ----
                for j in range(NZ):
                    js = slice(j * 512, (j + 1) * 512)
                    yb = pyacc.tile([128, 512], fp32, tag="yacc")
                    nc.tensor.matmul(
                        yb[:], lhsT=C_ch, rhs=S_b[:, js], start=True, stop=True
                    )
                    # in-place decay scale of the inter-chunk term
                    nc.vector.tensor_tensor(
                        out=yb[:].rearrange("p (h d) -> p h d", h=8),
                        in0=yb[:].rearrange("p (h d) -> p h d", h=8),
                        in1=ecb[:, j * 8 : (j + 1) * 8, :],
                        op=AX.mult,
                    )
                    for g in range(j * 2, j * 2 + 2):  # 4-head groups
                        pbc = pbcp.tile([128, 512], fp32, tag="bcast")
                        nc.tensor.matmul(
                            pbc[:], lhsT=bT88[:],
                            rhs=rhs88[:, g * 512 : (g + 1) * 512],
                            start=True, stop=True,
                        )
                        # exp from fp32 PSUM (precision), then kill the masked
                        # region's infs (exponent reaches ~240 there) with a
                        # finite min before the causal-zero multiply
                        mex = smpool.tile([128, 4, Q], bf16, tag="mex", bufs=2)
                        nc.scalar.activation(mex[:], pbc[:], ACT.Exp)
                        mexm = smpool.tile([128, 4, Q], bf16, tag="mexm", bufs=2)
                        nc.vector.tensor_scalar(
                            out=mexm[:], in0=mex[:], scalar1=1e12, scalar2=None,
                            op0=AX.min,
                        )
                        nc.vector.tensor_tensor(
                            out=mexm[:], in0=mexm[:], in1=gtm4[:], op=AX.mult
                        )
                        for hg in range(4):
                            h = g * 4 + hg
                            nc.tensor.matmul(
                                yb[:, (h - j * 8) * 64 : (h - j * 8 + 1) * 64],
                                lhsT=mexm[:, hg, :],
                                rhs=xst[:, h * 64 : (h + 1) * 64],
                                start=False, stop=(hg == 3 and g == j * 2 + 1),
                                skip_group_check=True,
                            )
                    # evacuate: ywork = yb + dxs
                    nc.vector.tensor_tensor(
                        out=ywork[:, js], in0=yb[:], in1=dxs[:, js], op=AX.add
                    )

                # ---- state update: S += Btok^T @ wXd (after decay) ----
                # per-slice S_b copies so next chunk's Y_inter matmuls start
                # as soon as their slice of the state is ready
                if pending is not None:
                    pending()  # previous chunk's deferred out-proj

                wxd_f = wxd[:].rearrange("p h d -> p (h d)")
                for j in range(NZ):
                    js = slice(j * 512, (j + 1) * 512)
                    pds = pyacc.tile([64, 512], fp32, tag="yacc")
                    nc.tensor.matmul(
                        pds[:], lhsT=btok[:], rhs=wxd_f[:, js],
                        start=True, stop=True,
                    )
                    nc.vector.tensor_tensor(
                        out=S_f[:, js], in0=S_f[:, js], in1=pds[:], op=AX.add
                    )
                    nc.vector.tensor_copy(S_b[:, js], S_f[:, js])

                if mid_cb is not None:
                    mid_cb()

                # ---- gate, then RMS stats ----
                nc.vector.tensor_tensor(
                    out=ywork[:], in0=ywork[:], in1=sz_list[c][:], op=AX.mult
                )
                sq = spool.tile([128, D_INNER], bf16, tag="dxs", bufs=1)
                ssum = smpool.tile([128, 1], fp32, tag="ssum")
                nc.scalar.activation(
                    sq[:], ywork[:], ACT.Square, accum_out=ssum[:],
                )
                vmean = smpool.tile([128, 1], fp32, tag="vmean")
                nc.vector.tensor_scalar(
                    out=vmean[:], in0=ssum[:], scalar1=1.0 / D_INNER,
                    scalar2=1e-5, op0=AX.mult, op1=AX.add,
                )
                # rsqrt on DVE (bit hack + 2 Newton steps) -- keeps the scalar
                # engine inside one activation table for the whole chunk loop
                rstd = smpool.tile([128, 1], fp32, tag="rstd")
                ri = rstd[:].bitcast(mybir.dt.int32)
                nc.vector.tensor_scalar(
                    out=ri, in0=vmean[:].bitcast(mybir.dt.int32),
                    scalar1=1, scalar2=None, op0=AX.arith_shift_right,
                )
                nc.vector.tensor_scalar(
                    out=ri, in0=ri, scalar1=-1, scalar2=0x5F3759DF,
                    op0=AX.mult, op1=AX.add,
                )
                rt = smpool.tile([128, 1], fp32, tag="rt")
                for _ in range(1):
                    nc.vector.tensor_tensor(out=rt[:], in0=rstd[:], in1=rstd[:], op=AX.mult)
                    nc.vector.tensor_tensor(out=rt[:], in0=rt[:], in1=vmean[:], op=AX.mult)
                    nc.vector.tensor_scalar(
                        out=rt[:], in0=rt[:], scalar1=-0.5, scalar2=1.5,
                        op0=AX.mult, op1=AX.add,
                    )
                    nc.vector.tensor_tensor(out=rstd[:], in0=rstd[:], in1=rt[:], op=AX.mult)

                # hoist next chunk's feed DMAs ahead of this chunk's tail
                if tail_feed is not None:
                    emit_feed(li, tail_feed)

                # ---- transpose ywork: ONE batched xbar DMA per chunk. The 3D out
                # lands channel c=p*12+j at [p, j, :]; wcb is host-permuted to match.
                ynt = spool.tile([128, 12, 128], bf16, tag="wxd")
                nc.sync.dma_start_transpose(out=ynt[:], in_=ywork[:])

                def out_tail():
                    # deferred out-proj: emitted inside the NEXT chunk so
                    # neither queue head-of-line blocks on it (the output DMA
                    # has no consumer, so its latency is free)
                    osb = spool.tile([128, D_MODEL], bf16, tag="osb", bufs=2)
                    for n2 in range(2):
                        po = pyacc.tile([128, 384], fp32, tag="yacc")
                        for j in range(12):
                            nc.tensor.matmul(
                                po[:],
                                lhsT=ynt[:, j, :],
                                rhs=wcb_sb[:, j, n2 * 384 : (n2 + 1) * 384],
                                start=(j == 0), stop=(j == 11),
                            )
                        # fold the RMS-norm scale into the PSUM evacuation
                        nc.vector.tensor_scalar(
                            out=osb[:, n2 * 384 : (n2 + 1) * 384], in0=po[:],
                            scalar1=rstd[:], scalar2=None, op0=AX.mult,
                        )
                    nc.sync.dma_start(
                        out=out.ap()[l0 + c * Q : l0 + (c + 1) * Q, :], in_=osb[:]
                    )
                return out_tail

            # software-pipelined emission: L-tile li+1's conv/dt matmuls are
            # interleaved between li's chunks so the in-order tensor/DVE queues
            # never drain at L-tile boundaries
            emit_head(0)
            emit_conv(0, range(MX))
            emit_bc_silu(0)
            emit_dt(0)
            emit_feed(0, 0)
            emit_feed(0, 1)
            def make_mid(li, c):
                if li + 1 >= NLT:
                    return None
                def cb():
                    if c == 0:
                        emit_head(li + 1)
                        emit_conv(li + 1, range(0, 3))
                    elif c == 1:
                        emit_conv(li + 1, range(3, 7))
                    elif c == 2:
                        emit_conv(li + 1, range(7, 10))
                    elif c == 3:
                        emit_conv(li + 1, range(10, MX))
                return cb

            pend = None
            for li in range(NLT):
                for c in range(NCH):
                    pend = emit_chunk(li, c,
                                      tail_feed=(c + 2 if c + 2 < NCH else None),
                                      mid_cb=make_mid(li, c), pending=pend)
                    if c == 3 and li + 1 < NLT:
                        emit_bc_silu(li + 1)
                        emit_dt(li + 1)
                        emit_feed(li + 1, 0)
                        emit_feed(li + 1, 1)
                del lctx[li]
            pend()  # final chunk's deferred out-proj evacuation

    nc.finalize()
    return nc


def _make_ind2():
    ind = np.zeros((88, H * Q), ml_dtypes.bfloat16)
    for h in range(H):
        ind[32 + h, h * Q : (h + 1) * Q] = 1.0
        ind[64 + h, h * Q : (h + 1) * Q] = 1.0
    return ind


def _prep_core_inputs(xb, p, flip):
    """Host-side preprocessing for one (branch, batch) core."""
    (in_w, conv_w, conv_b, dt_bias, A_log, Dp, norm_w, out_w, fus_half) = p
    x = xb[::-1] if flip else xb
    xT = np.ascontiguousarray(x.T).astype(ml_dtypes.bfloat16)
    wip = np.ascontiguousarray(in_w.T).astype(ml_dtypes.bfloat16)
    wcomb = (np.diag(norm_w.astype(np.float64)) @ out_w.T.astype(np.float64)
             @ fus_half.T.astype(np.float64)).astype(np.float32)
    wcb = wcomb.astype(ml_dtypes.bfloat16)
    cw = np.ascontiguousarray(conv_w[:, 0, :]).astype(np.float32)
    cb = conv_b.astype(np.float32)
    dtb = dt_bias.reshape(H, 1).astype(np.float32)
    apos = np.exp(A_log).reshape(H, 1).astype(np.float32)
    dsb = np.broadcast_to(np.repeat(Dp, HEADDIM)[None, :], (128, D_INNER))
    dsb = np.ascontiguousarray(dsb).astype(ml_dtypes.bfloat16)
    return {
        "xT": xT, "wip": wip, "wcb": wcb, "cw": cw, "cb": cb,
        "dtb": dtb, "apos": apos, "dsb": dsb, "ind2": _make_ind2(),
    }


def kernel(x, fus_w, fus_b,
           f_in_w, f_conv_w, f_conv_b, f_dt_bias, f_A_log, f_D, f_norm_w, f_out_w,
           b_in_w, b_conv_w, b_conv_b, b_dt_bias, b_A_log, b_D, b_norm_w, b_out_w):
    from concourse.bass_utils import run_bass_kernel_spmd

    if "nc" not in _CACHE:
        _CACHE["nc"] = _build_nc()
    nc = _CACHE["nc"]

    x = np.asarray(x, dtype=np.float32)
    fp = (f_in_w, f_conv_w, f_conv_b, f_dt_bias, f_A_log, f_D, f_norm_w, f_out_w,
          fus_w[:, :D_MODEL])
    bp = (b_in_w, b_conv_w, b_conv_b, b_dt_bias, b_A_log, b_D, b_norm_w, b_out_w,
          fus_w[:, D_MODEL:])
    fp = tuple(np.asarray(a) for a in fp)
    bp = tuple(np.asarray(a) for a in bp)

    in_maps = []
    for b in range(BATCH):
        in_maps.append(_prep_core_inputs(x[b], fp, flip=False))
    for b in range(BATCH):
        in_maps.append(_prep_core_inputs(x[b], bp, flip=True))

    res = run_bass_kernel_spmd(nc, in_maps, list(range(8)))
    out = np.empty((BATCH, SEQ, D_MODEL), np.float32)
    for b in range(BATCH):
        of = np.asarray(res.results[b]["out"], np.float32)
        ob = np.asarray(res.results[BATCH + b]["out"], np.float32)[::-1]
        out[b] = of + ob + np.asarray(fus_b, np.float32)[None, :]
    return out


# revision 83
# speedup vs baseline: 1.0238x; 1.0238x over previous
"""BiMamba (fwd+bwd Mamba2 + fusion Linear) Trainium2 kernel.

Sharding: 8 cores = 2 branches x 4 batches. Each core runs one full Mamba2
branch on one batch element via the chunked SSD formulation (chunk=128), with
the out-proj and fusion Linear folded into one matmul (W_comb). Host flips x
for the backward branch and sums the two per-branch partial outputs.

Optimizations over the first working version (1.265ms -> ~1.09ms):
- Y_inter and the per-head Y_intra matmuls accumulate into one PSUM bank ring
  (in-place decay scale between accumulation groups) instead of separate
  PSUM tiles + DVE adds; the state-update and out-proj matmuls reuse the ring.
- rsqrt for the RMS norm runs on the DVE (bit hack + Newton), and softplus/ln
  stay paired with exp, so the scalar engine needs ~2 act-table loads per
  L-tile instead of 3 per chunk (was 124us of table thrash).
- The RMS-norm scale is folded into the out-proj PSUM evacuation (tensor_scalar
  with the per-token rstd), removing a full-width normalize pass.
- Conv tiles are transposed to token layout pre-SiLU as soon as each conv
  m-tile finishes (overlapping the conv phase); SiLU is applied per-chunk on
  the transposed tile, removing the L-tile-start serialization.
- Software-pipelined emission: the next L-tile's in_proj/conv/dt matmuls are
  emitted between the current L-tile's chunks (engine queues are in-order, so
  emission order controls overlap); feeds run two chunks ahead through a
  triple-buffered mask-operand ring.
- Chunk-invariant pieces (decay transposes, causal-masked G^T, B token layout)
  are precomputed one chunk ahead, off the state-recurrence spine.

Engine notes (hardware-measured): GpSimd elementwise offload is a trap (Q7
SBUF traffic slows DVE/Scalar ~1.5x); a single fp32 matmul inserted in the
bf16 stream costs ~70us in PE mode switches; scalar-engine ops run ~2x the
documented cycle model on this part.
"""

import sys

sys.path.insert(0, "/opt/trn_rl_repo")

import numpy as np
import ml_dtypes

D_MODEL = 768
D_STATE = 64
D_CONV = 4
D_INNER = 1536
HEADDIM = 64
H = 24
CONV_DIM = D_INNER + 2 * D_STATE  # 1664
D_IN_PROJ = 2 * D_INNER + 2 * D_STATE + H  # 3224
BATCH, SEQ = 4, 4096

LT = 512  # L-tile
NLT = SEQ // LT  # 8
Q = 128  # chunk
NCH = LT // Q  # chunks per L-tile
KT = D_MODEL // 128  # 6 k-tiles
MX = 13  # xBC m-tiles (1664/128)
NZ = D_INNER // 512  # 3 z slices
BF = "bfloat16"

_CACHE = {}


def _build_nc():
    import concourse.bass as bass
    import concourse.bacc as bacc
    import concourse.mybir as mybir
    from concourse.tile import TileContext
    from concourse.masks import make_identity

    fp32 = mybir.dt.float32
    bf16 = mybir.dt.bfloat16
    AX = mybir.AluOpType
    ACT = mybir.ActivationFunctionType

    nc = bacc.Bacc("TRN2", debug=False, num_devices=8)

    xT = nc.declare_dram_parameter("xT", [D_MODEL, SEQ], bf16, isOutput=False)
    wip = nc.declare_dram_parameter("wip", [D_MODEL, D_IN_PROJ], bf16, isOutput=False)
    wcb = nc.declare_dram_parameter("wcb", [D_INNER, D_MODEL], bf16, isOutput=False)
    cw = nc.declare_dram_parameter("cw", [CONV_DIM, D_CONV], fp32, isOutput=False)
    cb = nc.declare_dram_parameter("cb", [CONV_DIM], fp32, isOutput=False)
    dtb = nc.declare_dram_parameter("dtb", [H, 1], fp32, isOutput=False)
    apos = nc.declare_dram_parameter("apos", [H, 1], fp32, isOutput=False)
    dsb = nc.declare_dram_parameter("dsb", [128, D_INNER], bf16, isOutput=False)
    ind2 = nc.declare_dram_parameter("ind2", [88, H * Q], bf16, isOutput=False)
    out = nc.declare_dram_parameter("out", [SEQ, D_MODEL], bf16, isOutput=True)

    with TileContext(nc) as tc:
        with (
            tc.tile_pool(name="const", bufs=1) as cpool,
            tc.tile_pool(name="xt", bufs=2) as xtpool,
            tc.tile_pool(name="work", bufs=2) as wpool,
            tc.tile_pool(name="conv", bufs=2) as convpool,
            tc.tile_pool(name="convin", bufs=1) as cinpool,
            tc.tile_pool(name="ssd", bufs=2) as spool,
            tc.tile_pool(name="ssdsm", bufs=3) as smpool,
            tc.tile_pool(name="state", bufs=1) as statepool,
            tc.tile_pool(name="pbig", bufs=2, space="PSUM") as pbig,
            tc.tile_pool(name="pbc", bufs=2, space="PSUM") as pbcp,
            tc.tile_pool(name="psm", bufs=1, space="PSUM") as psm,
            tc.tile_pool(name="pyacc", bufs=3, space="PSUM") as pyacc,
        ):
            # ---- constants ----
            wip_sb = cpool.tile([128, KT, D_IN_PROJ], bf16, tag="wip")
            nc.sync.dma_start(
                out=wip_sb[:], in_=wip.ap().rearrange("(k p) m -> p k m", p=128)
            )
            wcb_sb = cpool.tile([128, 12, D_MODEL], bf16, tag="wcb")
            nc.sync.dma_start(
                out=wcb_sb[:], in_=wcb.ap().rearrange("(j p) m -> p j m", p=128)
            )
            cw_sb = cpool.tile([128, MX, D_CONV], fp32, tag="cw")
            nc.sync.dma_start(
                out=cw_sb[:], in_=cw.ap().rearrange("(a p) k -> p a k", p=128)
            )
            cb_sb = cpool.tile([128, MX], fp32, tag="cb")
            nc.sync.dma_start(
                out=cb_sb[:], in_=cb.ap().rearrange("(a p) -> p a", p=128)
            )
            dtb_sb = cpool.tile([H, 1], fp32, tag="dtb")
            nc.sync.dma_start(out=dtb_sb[:], in_=dtb.ap())
            apos_sb = cpool.tile([H, 1], fp32, tag="apos")
            nc.sync.dma_start(out=apos_sb[:], in_=apos.ap())
            dsb_sb = cpool.tile([128, D_INNER], bf16, tag="dsb")
            nc.sync.dma_start(out=dsb_sb[:], in_=dsb.ap())
            rhs88p = []
            for pi in range(3):
                r8 = cpool.tile([88, H * Q], bf16, tag=f"rhs88_{pi}")
                nc.sync.dma_start(out=r8[:], in_=ind2.ap())
                rhs88p.append(r8)

            ident_f64 = cpool.tile([64, 64], fp32, tag="idf64")
            make_identity(nc, ident_f64[:])
            ident_b128 = cpool.tile([128, 128], bf16, tag="idb")
            make_identity(nc, ident_b128[:])
            bT88p = []
            for pi in range(3):
                b8 = cpool.tile([88, 128], bf16, tag=f"bT88_{pi}")
                nc.gpsimd.memset(b8[:], 0.0)
                nc.gpsimd.memset(b8[0:2, :], -1.0)
                bT88p.append(b8)
            pones2 = cpool.tile([2, 64], bf16, tag="pones2")
            nc.gpsimd.memset(pones2[:], 1.0)
            zer24 = cpool.tile([H, 128], bf16, tag="zer24")
            nc.gpsimd.memset(zer24[:], 0.0)

            # ---- loop-carried state ----
            S_f = statepool.tile([64, H * 64], fp32, tag="Sf")
            nc.vector.memset(S_f[:], 0.0)
            S_b = statepool.tile([64, H * 64], bf16, tag="Sb")
            nc.vector.memset(S_b[:], 0.0)

            halo = [
                convpool.tile([128, 3], bf16, tag=f"halo{m}", name=f"halo{m}")
                for m in range(MX)
            ]
            for m in range(MX):
                nc.vector.memset(halo[m][:], 0.0)

            lctx = {}

            def emit_head(li):
                l0 = li * LT
                xtt = xtpool.tile([128, KT, LT], bf16, tag="xtt")
                nc.sync.dma_start(
                    out=xtt[:],
                    in_=xT.ap().rearrange("(k p) l -> p k l", p=128)[:, :, l0 : l0 + LT],
                )
                # bufs=1: each conv tile is consumed (transposed / B-C copied)
                # before the next L-tile's conv overwrites it
                co_all = convpool.tile([128, MX, LT], bf16, tag="co", bufs=1)
                # token-major x (pre-SiLU); filled by per-m transposes during conv
                xstLT = wpool.tile([128, NCH, D_INNER], bf16, tag="xstLT", bufs=2)
                lctx[li] = {"xtt": xtt, "co": co_all, "xstLT": xstLT}

            def emit_conv(li, ms):
                ctx = lctx[li]
                xtt, co_all = ctx["xtt"], ctx["co"]
                for m in ms:
                    ps = pbig.tile([128, LT], fp32, tag="big")
                    for k in range(KT):
                        nc.tensor.matmul(
                            ps[:],
                            lhsT=wip_sb[:, k, D_INNER + m * 128 : D_INNER + (m + 1) * 128],
                            rhs=xtt[:, k, :],
                            start=(k == 0),
                            stop=(k == KT - 1),
                        )
                    xin = cinpool.tile([128, LT + 3], bf16, tag="xin", bufs=2)
                    nc.any.tensor_copy(xin[:, 0:3], halo[m][:])
                    nc.any.tensor_copy(xin[:, 3 : LT + 3], ps[:])
                    nh = convpool.tile([128, 3], bf16, tag=f"halo{m}")
                    nc.any.tensor_copy(nh[:], xin[:, LT : LT + 3])
                    halo[m] = nh
                    # conv as 4 independent TS products (4x DVE mode) + add tree
                    # (all on DVE: cross-engine splits of this chain cause SBUF
                    # contention that slows both engines)
                    co = co_all[:, m, :]
                    t0 = cinpool.tile([128, LT], bf16, tag="t0", bufs=2)
                    nc.vector.tensor_scalar(
                        out=t0[:], in0=xin[:, 3 : LT + 3],
                        scalar1=cw_sb[:, m, 3:4], scalar2=cb_sb[:, m : m + 1],
                        op0=AX.mult, op1=AX.add,
                    )
                    t1 = cinpool.tile([128, LT], bf16, tag="t1", bufs=2)
                    nc.vector.tensor_scalar(
                        out=t1[:], in0=xin[:, 2 : LT + 2],
                        scalar1=cw_sb[:, m, 2:3], scalar2=None, op0=AX.mult,
                    )
                    nc.vector.tensor_tensor(out=t0[:], in0=t0[:], in1=t1[:], op=AX.add)
                    nc.vector.tensor_scalar(
                        out=t1[:], in0=xin[:, 1 : LT + 1],
                        scalar1=cw_sb[:, m, 1:2], scalar2=None, op0=AX.mult,
                    )
                    t2 = cinpool.tile([128, LT], bf16, tag="t2", bufs=2)
                    nc.vector.tensor_scalar(
                        out=t2[:], in0=xin[:, 0:LT],
                        scalar1=cw_sb[:, m, 0:1], scalar2=None, op0=AX.mult,
                    )
                    nc.vector.tensor_tensor(out=t1[:], in0=t1[:], in1=t2[:], op=AX.add)
                    nc.vector.tensor_tensor(out=co, in0=t0[:], in1=t1[:], op=AX.add)
                    # transpose this conv tile to token layout NOW (pre-SiLU) so
                    # the xbar DMAs overlap the remaining conv work; SiLU is
                    # applied per-chunk on the transposed tile instead
                    if m < 12:
                        nc.sync.dma_start_transpose(
                            out=ctx["xstLT"][:, :, m * 128 : (m + 1) * 128],
                            in_=co,
                        )

            def emit_bc_silu(li):
                # B/C conv tile SiLU (channel layout) + per-chunk SiLU on
                # token-major x + z-gate matmuls/SiLU: one contiguous SiLU
                # block per L-tile keeps the act-table switches at ~2/L-tile
                ctx = lctx[li]
                xtt, co_all = ctx["xtt"], ctx["co"]
                xstLT = ctx["xstLT"]
                nc.scalar.activation(co_all[:, 12, :], co_all[:, 12, :], ACT.Silu)
                Bt = convpool.tile([64, LT], bf16, tag="Bt", bufs=1)
                nc.any.tensor_copy(Bt[:], co_all[0:64, 12, :])
                Ct = convpool.tile([64, LT], bf16, tag="Ct", bufs=1)
                nc.sync.dma_start(out=Ct[:], in_=co_all[64:128, 12, :])
                for c4 in range(NCH):
                    nc.scalar.activation(
                        xstLT[:, c4, :], xstLT[:, c4, :], ACT.Silu
                    )
                sz_list = []
                for c4 in range(NCH):
                    c4s = slice(c4 * Q, (c4 + 1) * Q)
                    sz_c = wpool.tile([128, D_INNER], bf16, tag="szlt", bufs=4)
                    for j in range(NZ):
                        pz = pbig.tile([128, 512], fp32, tag="big")
                        for k in range(KT):
                            nc.tensor.matmul(
                                pz[:],
                                lhsT=xtt[:, k, c4s],
                                rhs=wip_sb[:, k, j * 512 : (j + 1) * 512],
                                start=(k == 0), stop=(k == KT - 1),
                            )
                        nc.scalar.activation(
                            sz_c[:, j * 512 : (j + 1) * 512], pz[:], ACT.Silu
                        )
                    sz_list.append(sz_c)
                ctx.update(Bt=Bt, Ct=Ct, sz_list=sz_list)

            def emit_dt(li):
                ctx = lctx[li]
                xtt = ctx["xtt"]
                # ---- in_proj: dt part (channel layout, 24 rows) ----
                psdt = pbig.tile([H, LT], fp32, tag="big")
                for k in range(KT):
                    nc.tensor.matmul(
                        psdt[:],
                        lhsT=wip_sb[:, k, D_INNER + CONV_DIM : D_IN_PROJ],
                        rhs=xtt[:, k, :],
                        start=(k == 0),
                        stop=(k == KT - 1),
                    )
                # dt = softplus(psdt + dt_bias) via exp + ln1p (both in one act table)
                art0 = wpool.tile([H, LT], fp32, tag="art0", bufs=1)
                nc.scalar.activation(art0[:], psdt[:], ACT.Exp, bias=dtb_sb[:])
                dtt = wpool.tile([H, LT], fp32, tag="dtt", bufs=1)
                nc.scalar.activation(dtt[:], art0[:], ACT.Ln, bias=1.0)
                # a*dt and its within-chunk cumsum
                art = wpool.tile([H, LT], fp32, tag="art", bufs=1)
                nc.vector.tensor_scalar(
                    out=art[:], in0=dtt[:], scalar1=apos_sb[:],
                    scalar2=None, op0=AX.mult,
                )
                cnt = wpool.tile([H, LT], fp32, tag="cnt", bufs=1)
                for c in range(NCH):
                    cs = slice(c * Q, (c + 1) * Q)
                    nc.vector.tensor_tensor_scan(
                        out=cnt[:, cs],
                        data0=art[:, cs],
                        data1=zer24[:],
                        initial=0.0,
                        op0=AX.add,
                        op1=AX.add,
                    )
                # ln(dt) head-major for the mask-bias matmul
                lnb = wpool.tile([H, LT], fp32, tag="lnb", bufs=1)
                nc.scalar.activation(lnb[:], dtt[:], ACT.Ln)
                nc.vector.tensor_tensor(out=lnb[:], in0=lnb[:], in1=cnt[:], op=AX.add)

                hi24 = wpool.tile([H, LT], bf16, tag="hi24", bufs=1)
                nc.any.tensor_copy(hi24[:], cnt[:])
                lo24 = wpool.tile([H, LT], bf16, tag="lo24", bufs=1)
                nc.vector.tensor_sub(lo24[:], cnt[:], hi24[:])
                ctx.update(dtt=dtt, cnt=cnt, lnb=lnb, hi24=hi24, lo24=lo24)

                # ---- per-L-tile precompute of all chunk-invariant SSD pieces
                # (off the per-chunk state-recurrence spine) ----
                # wv for all chunks: exp(cnt - chunk_end_cnt)
                cend = wpool.tile([H, NCH], fp32, tag="cend", bufs=1)
                nc.vector.tensor_copy(
                    cend[:], cnt[:].rearrange("p (c q) -> p c q", c=NCH)[:, :, Q - 1]
                )
                wvLT = wpool.tile([H, LT], fp32, tag="wvLT", bufs=1)
                nc.vector.tensor_tensor(
                    out=wvLT[:].rearrange("p (c q) -> p c q", c=NCH),
                    in0=cnt[:].rearrange("p (c q) -> p c q", c=NCH),
                    in1=cend[:].unsqueeze(2).broadcast_to([H, NCH, Q]),
                    op=AX.subtract,
                )
                nc.scalar.activation(wvLT[:], wvLT[:], ACT.Exp)
                # stacked [wdt ; pad ; cnt] (64, LT): one build for all chunks
                stkLT = wpool.tile([64, LT], fp32, tag="stkLT", bufs=1)
                nc.gpsimd.memset(stkLT[:], 0.0)
                nc.vector.tensor_tensor(
                    out=stkLT[0:H, :], in0=dtt[:], in1=wvLT[:], op=AX.mult
                )
                nc.vector.tensor_copy(stkLT[32 : 32 + H, :], cnt[:])
                ctx.update(stkLT=stkLT, wdtTs={}, expcums={}, gtm4s={}, btoks={})

            def emit_pre(li, c):
                # chunk-invariant SSD pieces for chunk c, emitted one chunk
                # ahead (pipelined off the state-recurrence spine)
                ctx = lctx[li]
                Bt, Ct, stkLT = ctx["Bt"], ctx["Ct"], ctx["stkLT"]
                cs = slice(c * Q, (c + 1) * Q)
                pwc = psm.tile([128, 64], fp32, tag="sm2")
                nc.tensor.transpose(pwc[:], stkLT[:, cs], ident_f64[:])
                wdtT = smpool.tile([128, H], bf16, tag="wdtT", bufs=4)
                nc.vector.tensor_copy(wdtT[:], pwc[:, 0:H])
                expcum = smpool.tile([128, H], fp32, tag="expcum", bufs=4)
                nc.scalar.activation(
                    expcum[:], pwc[:, 32 : 32 + H], ACT.Exp, scale=-1.0
                )
                # G^T with causal mask + 4-head broadcast
                pgt = psm.tile([128, Q], fp32, tag="sm2")
                nc.tensor.matmul(
                    pgt[:], lhsT=Bt[:, cs], rhs=Ct[:, cs], start=True, stop=True
                )
                gtc = smpool.tile([128, Q], bf16, tag="gtc", bufs=4)
                nc.vector.tensor_copy(gtc[:], pgt[:])
                gtm4 = smpool.tile([128, 4, Q], bf16, tag="gtm4", bufs=4)
                nc.gpsimd.affine_select(
                    out=gtm4[:],
                    in_=gtc[:].unsqueeze(1).broadcast_to([128, 4, Q]),
                    compare_op=AX.is_ge, fill=0.0,
                    base=0, pattern=[[0, 4], [1, Q]], channel_multiplier=-1,
                )
                # B token layout
                pbt = psm.tile([128, 64], bf16, tag="sm2")
                nc.tensor.transpose(pbt[:], Bt[:, cs], ident_b128[0:64, 0:64])
                btok = smpool.tile([128, 64], bf16, tag="btok", bufs=4)
                nc.vector.tensor_copy(btok[:], pbt[:])
                ctx["wdtTs"][c] = wdtT; ctx["expcums"][c] = expcum
                ctx["gtm4s"][c] = gtm4; ctx["btoks"][c] = btok

            def emit_feed(li, c):
                ctx = lctx[li]
                lnb, hi24, lo24 = ctx["lnb"], ctx["hi24"], ctx["lo24"]
                cs = slice(c * Q, (c + 1) * Q)
                rhs88 = rhs88p[(li * NCH + c) % 3]
                bT88 = bT88p[(li * NCH + c) % 3]
                h4 = rhs88[0:2, :].rearrange("p (h t) -> p h t", h=H)
                nc.sync.dma_start(out=h4[0:1, :, :], in_=hi24[:, cs])
                nc.sync.dma_start(out=h4[1:2, :, :], in_=lo24[:, cs])
                # bias rows (bf16 hi/lo of ln(dt)+cumneg), head-major
                bhi = smpool.tile([H, 128], bf16, tag="bhi")
                nc.any.tensor_copy(bhi[:], lnb[:, cs])
                blo = smpool.tile([H, 128], bf16, tag="blo")
                nc.vector.tensor_sub(blo[:], lnb[:, cs], bhi[:])
                nc.sync.dma_start(out=bT88[32 : 32 + H, :], in_=bhi[:])
                nc.sync.dma_start(out=bT88[64 : 64 + H, :], in_=blo[:])

            def emit_chunk(li, c, tail_feed=None, mid_cb=None, pending=None):
                ctx = lctx[li]
                Bt, Ct, sz_list = ctx["Bt"], ctx["Ct"], ctx["sz_list"]
                dtt, cnt = ctx["dtt"], ctx["cnt"]
                l0 = li * LT
                cs = slice(c * Q, (c + 1) * Q)
                lend = c * Q + Q - 1
                rhs88 = rhs88p[(li * NCH + c) % 3]
                bT88 = bT88p[(li * NCH + c) % 3]
                xst = ctx["xstLT"][:, c, :]
                B_ch = Bt[:, cs]
                C_ch = Ct[:, cs]

                emit_pre(li, c)
                wdtT = ctx["wdtTs"][c]
                expcum = ctx["expcums"][c]
                gtm4 = ctx["gtm4s"][c]
                btok = ctx["btoks"][c]

                # chunk decay factors (64, 24) from the feed rows
                pcd = psm.tile([64, H], fp32, tag="sm2")
                nc.tensor.matmul(
                    pcd[:], lhsT=pones2[:], rhs=rhs88[0:2, :].rearrange("p (h t) -> p h t", h=H)[:, :, Q - 1],
                    start=True, stop=True,
                )
                cdec = smpool.tile([64, H], fp32, tag="cdec")
                nc.scalar.activation(cdec[:], pcd[:], ACT.Exp, scale=-1.0)

                # wxd = wdt * x (token layout). NOTE: GpSimd elementwise is a
                # trap — Q7 SBUF traffic contends with the DVE/Scalar ports and
                # slows every other engine ~1.5x. Keep on DVE.
                wdt_b = wdtT[:].unsqueeze(2).broadcast_to([128, H, 64])
                wxd = spool.tile([128, H, 64], bf16, tag="wxd")
                nc.vector.tensor_tensor(
                    out=wxd[:],
                    in0=xst.rearrange("p (h d) -> p h d", h=H),
                    in1=wdt_b, op=AX.mult,
                )
                # state decay (in place)
                cdb = cdec[:].unsqueeze(2).broadcast_to([64, H, 64])
                nc.vector.tensor_tensor(
                    out=S_f[:].rearrange("p (h d) -> p h d", h=H),
                    in0=S_f[:].rearrange("p (h d) -> p h d", h=H),
                    in1=cdb, op=AX.mult,
                )

                # dxs = D * x (skip term)
                dxs = spool.tile([128, D_INNER], bf16, tag="dxs", bufs=1)
                nc.vector.tensor_tensor(out=dxs[:], in0=xst, in1=dsb_sb[:], op=AX.mult)

                if pending is not None:
                    pending()  # previous chunk's deferred out-proj

                ywork = spool.tile([128, D_INNER], bf16, tag="ywork")
                ecb = expcum[:].unsqueeze(2).broadcast_to([128, H, 64])

                # ---- Y accumulation: inter (scaled in place) + intra heads ----
                for j in range(NZ):
                    js = slice(j * 512, (j + 1) * 512)
                    yb = pyacc.tile([128, 512], fp32, tag="yacc")
                    nc.tensor.matmul(
                        yb[:], lhsT=C_ch, rhs=S_b[:, js], start=True, stop=True
                    )
                    # in-place decay scale of the inter-chunk term
                    nc.vector.tensor_tensor(
                        out=yb[:].rearrange("p (h d) -> p h d", h=8),
                        in0=yb[:].rearrange("p (h d) -> p h d", h=8),
                        in1=ecb[:, j * 8 : (j + 1) * 8, :],
                        op=AX.mult,
                    )
                    for g in range(j * 2, j * 2 + 2):  # 4-head groups
                        pbc = pbcp.tile([128, 512], fp32, tag="bcast")
                        nc.tensor.matmul(
                            pbc[:], lhsT=bT88[:],
                            rhs=rhs88[:, g * 512 : (g + 1) * 512],
                            start=True, stop=True,
                        )
                        # exp from fp32 PSUM (precision), then kill the masked
                        # region's infs (exponent reaches ~240 there) with a
                        # finite min before the causal-zero multiply
                        mex = smpool.tile([128, 4, Q], bf16, tag="mex", bufs=2)
                        nc.scalar.activation(mex[:], pbc[:], ACT.Exp)
                        mexm = smpool.tile([128, 4, Q], bf16, tag="mexm", bufs=2)
                        nc.vector.tensor_scalar(
                            out=mexm[:], in0=mex[:], scalar1=1e12, scalar2=None,
                            op0=AX.min,
                        )
                        nc.vector.tensor_tensor(
                            out=mexm[:], in0=mexm[:], in1=gtm4[:], op=AX.mult
                        )
                        for hg in range(4):
                            h = g * 4 + hg
                            nc.tensor.matmul(
                                yb[:, (h - j * 8) * 64 : (h - j * 8 + 1) * 64],
                                lhsT=mexm[:, hg, :],
                                rhs=xst[:, h * 64 : (h + 1) * 64],
                                start=False, stop=(hg == 3 and g == j * 2 + 1),
                                skip_group_check=True,
                            )
                    # evacuate: ywork = yb + dxs
                    nc.vector.tensor_tensor(
                        out=ywork[:, js], in0=yb[:], in1=dxs[:, js], op=AX.add
                    )

                # ---- state update: S += Btok^T @ wXd (after decay) ----
                # per-slice S_b copies so next chunk's Y_inter matmuls start
                # as soon as their slice of the state is ready
                wxd_f = wxd[:].rearrange("p h d -> p (h d)")
                for j in range(NZ):
                    js = slice(j * 512, (j + 1) * 512)
                    pds = pyacc.tile([64, 512], fp32, tag="yacc")
                    nc.tensor.matmul(
                        pds[:], lhsT=btok[:], rhs=wxd_f[:, js],
                        start=True, stop=True,
                    )
                    nc.vector.tensor_tensor(
                        out=S_f[:, js], in0=S_f[:, js], in1=pds[:], op=AX.add
                    )
                    nc.vector.tensor_copy(S_b[:, js], S_f[:, js])

                if mid_cb is not None:
                    mid_cb()

                # ---- gate, then RMS stats ----
                nc.vector.tensor_tensor(
                    out=ywork[:], in0=ywork[:], in1=sz_list[c][:], op=AX.mult
                )
                sq = spool.tile([128, D_INNER], bf16, tag="dxs", bufs=1)
                ssum = smpool.tile([128, 1], fp32, tag="ssum")
                nc.scalar.activation(
                    sq[:], ywork[:], ACT.Square, accum_out=ssum[:],
                )
                vmean = smpool.tile([128, 1], fp32, tag="vmean")
                nc.vector.tensor_scalar(
                    out=vmean[:], in0=ssum[:], scalar1=1.0 / D_INNER,
                    scalar2=1e-5, op0=AX.mult, op1=AX.add,
                )
                # rsqrt on DVE (bit hack + 2 Newton steps) -- keeps the scalar
                # engine inside one activation table for the whole chunk loop
                rstd = smpool.tile([128, 1], fp32, tag="rstd")
                ri = rstd[:].bitcast(mybir.dt.int32)
                nc.vector.tensor_scalar(
                    out=ri, in0=vmean[:].bitcast(mybir.dt.int32),
                    scalar1=1, scalar2=None, op0=AX.arith_shift_right,
                )
                nc.vector.tensor_scalar(
                    out=ri, in0=ri, scalar1=-1, scalar2=0x5F3759DF,
                    op0=AX.mult, op1=AX.add,
                )
                rt = smpool.tile([128, 1], fp32, tag="rt")
                for _ in range(1):
                    nc.vector.tensor_tensor(out=rt[:], in0=rstd[:], in1=rstd[:], op=AX.mult)
                    nc.vector.tensor_tensor(out=rt[:], in0=rt[:], in1=vmean[:], op=AX.mult)
                    nc.vector.tensor_scalar(
                        out=rt[:], in0=rt[:], scalar1=-0.5, scalar2=1.5,
                        op0=AX.mult, op1=AX.add,
                    )
                    nc.vector.tensor_tensor(out=rstd[:], in0=rstd[:], in1=rt[:], op=AX.mult)

                # hoist next chunk's feed DMAs ahead of this chunk's tail
                if tail_feed is not None:
                    emit_feed(li, tail_feed)

                # ---- transpose ywork: ONE batched xbar DMA per chunk. The 3D out
                # lands channel c=p*12+j at [p, j, :]; wcb is host-permuted to match.
                ynt = spool.tile([128, 12, 128], bf16, tag="wxd")
                nc.sync.dma_start_transpose(out=ynt[:], in_=ywork[:])

                def out_tail():
                    # deferred out-proj: emitted inside the NEXT chunk so
                    # neither queue head-of-line blocks on it (the output DMA
                    # has no consumer, so its latency is free)
                    osb = spool.tile([128, D_MODEL], bf16, tag="osb", bufs=2)
                    for n2 in range(2):
                        po = pyacc.tile([128, 384], fp32, tag="yacc")
                        for j in range(12):
                            nc.tensor.matmul(
                                po[:],
                                lhsT=ynt[:, j, :],
                                rhs=wcb_sb[:, j, n2 * 384 : (n2 + 1) * 384],
                                start=(j == 0), stop=(j == 11),
                            )
                        # fold the RMS-norm scale into the PSUM evacuation
                        nc.vector.tensor_scalar(
                            out=osb[:, n2 * 384 : (n2 + 1) * 384], in0=po[:],
                            scalar1=rstd[:], scalar2=None, op0=AX.mult,
                        )
                    nc.sync.dma_start(
                        out=out.ap()[l0 + c * Q : l0 + (c + 1) * Q, :], in_=osb[:]
                    )
                return out_tail

            # software-pipelined emission: L-tile li+1's conv/dt matmuls are
            # interleaved between li's chunks so the in-order tensor/DVE queues
            # never drain at L-tile boundaries
            emit_head(0)
            emit_conv(0, range(MX))
            emit_bc_silu(0)
            emit_dt(0)
            emit_feed(0, 0)
            emit_feed(0, 1)
            def make_mid(li, c):
                if li + 1 >= NLT:
                    return None
                def cb():
                    if c == 0:
                        emit_head(li + 1)
                        emit_conv(li + 1, range(0, 3))
                    elif c == 1:
                        emit_conv(li + 1, range(3, 7))
                    elif c == 2:
                        emit_conv(li + 1, range(7, 10))
                    elif c == 3:
                        emit_conv(li + 1, range(10, MX))
                return cb

            pend = None
            for li in range(NLT):
                for c in range(NCH):
                    pend = emit_chunk(li, c,
                                      tail_feed=(c + 2 if c + 2 < NCH else None),
                                      mid_cb=make_mid(li, c), pending=pend)
                    if c == 3 and li + 1 < NLT:
                        emit_bc_silu(li + 1)
                        emit_dt(li + 1)
                        emit_feed(li + 1, 0)
                        emit_feed(li + 1, 1)
                del lctx[li]
            pend()  # final chunk's deferred out-proj evacuation

    nc.finalize()
    return nc


def _make_ind2():
    ind = np.zeros((88, H * Q), ml_dtypes.bfloat16)
    for h in range(H):
        ind[32 + h, h * Q : (h + 1) * Q] = 1.0
        ind[64 + h, h * Q : (h + 1) * Q] = 1.0
    return ind


def _prep_core_inputs(xb, p, flip):
    """Host-side preprocessing for one (branch, batch) core."""
    (in_w, conv_w, conv_b, dt_bias, A_log, Dp, norm_w, out_w, fus_half) = p
    x = xb[::-1] if flip else xb
    xT = np.ascontiguousarray(x.T).astype(ml_dtypes.bfloat16)
    wip = np.ascontiguousarray(in_w.T).astype(ml_dtypes.bfloat16)
    wcomb = (np.diag(norm_w.astype(np.float64)) @ out_w.T.astype(np.float64)
             @ fus_half.T.astype(np.float64)).astype(np.float32)
    wcb = wcomb.astype(ml_dtypes.bfloat16)
    cw = np.ascontiguousarray(conv_w[:, 0, :]).astype(np.float32)
    cb = conv_b.astype(np.float32)
    dtb = dt_bias.reshape(H, 1).astype(np.float32)
    apos = np.exp(A_log).reshape(H, 1).astype(np.float32)
    dsb = np.broadcast_to(np.repeat(Dp, HEADDIM)[None, :], (128, D_INNER))
    dsb = np.ascontiguousarray(dsb).astype(ml_dtypes.bfloat16)
    return {
        "xT": xT, "wip": wip, "wcb": wcb, "cw": cw, "cb": cb,
        "dtb": dtb, "apos": apos, "dsb": dsb, "ind2": _make_ind2(),
    }


def kernel(x, fus_w, fus_b,
           f_in_w, f_conv_w, f_conv_b, f_dt_bias, f_A_log, f_D, f_norm_w, f_out_w,
           b_in_w, b_conv_w, b_conv_b, b_dt_bias, b_A_log, b_D, b_norm_w, b_out_w):
    from concourse.bass_utils import run_bass_kernel_spmd

    if "nc" not in _CACHE:
        _CACHE["nc"] = _build_nc()
    nc = _CACHE["nc"]

    x = np.asarray(x, dtype=np.float32)
    fp = (f_in_w, f_conv_w, f_conv_b, f_dt_bias, f_A_log, f_D, f_norm_w, f_out_w,
          fus_w[:, :D_MODEL])
    bp = (b_in_w, b_conv_w, b_conv_b, b_dt_bias, b_A_log, b_D, b_norm_w, b_out_w,
          fus_w[:, D_MODEL:])
    fp = tuple(np.asarray(a) for a in fp)
    bp = tuple(np.asarray(a) for a in bp)

    in_maps = []
    for b in range(BATCH):
        in_maps.append(_prep_core_inputs(x[b], fp, flip=False))
    for b in range(BATCH):
        in_maps.append(_prep_core_inputs(x[b], bp, flip=True))

    res = run_bass_kernel_spmd(nc, in_maps, list(range(8)))
    out = np.empty((BATCH, SEQ, D_MODEL), np.float32)
    for b in range(BATCH):
        of = np.asarray(res.results[b]["out"], np.float32)
        ob = np.asarray(res.results[BATCH + b]["out"], np.float32)[::-1]
        out[b] = of + ob + np.asarray(fus_b, np.float32)[None, :]
    return out
